# revision 16
# baseline (speedup 1.0000x reference)
"""Trainium2 Bass kernel for nn_RecurrentGCN (TGCN cell + MLP head, output = y[2]).

The reference network returns y[2] — a single [1]-shaped value that depends only
on node 2's GCN aggregation.  With H0 = 0 the r-gate branch (Wr/br/Lr_*) and the
bottom halves of Lz_W/Lh_W are multiplied by zero, so the live computation is:

    deg[n]   = 1 + #(dst == n)                     (self loops add 1)
    g        = dinv2 * ( sum_{e: dst[e]==2} dinv[src[e]] * x[src[e]]
                         + dinv2 * x[2] )          with dinv = rsqrt(deg)
    cz = g @ Wz + bz ;  ch = g @ Wh + bh
    Z  = sigmoid(cz @ Lz_W[:64] + Lz_b) ; Ht = tanh(ch @ Lh_W[:64] + Lh_b)
    h  = (1 - Z) * Ht
    y  = relu(h) @ W1 + b1  -> BN(eval) -> relu -> @ W2 + b2

Device work: the 1.6M-entry dst array (the memory-bound part) is sharded across
the 8 NeuronCores; each core counts occurrences of the candidate node set
(node 2 + unique sources of its in-edges, baked as immediates) over its shard
with DVE is_equal+accumulate ops and ACT abs/relu indicator ops.  Partial counts
are summed on-chip (AllReduce), then every core runs the small dense epilogue
(host pre-folds Az=Wz@Lz[:64], BN into W1, and 1-sigmoid(x)=sigmoid(-x));
core 0's output is returned.
"""

import numpy as np

DEBUG = False

N = 100000
E = 1600000
HD = 64
BN_EPS = 1e-5
NCORES = 8
PART = 128
FREE = 1564                      # 128*1564 = 200192 >= E/8, per-core shard
SHARD = PART * FREE
PAD_DST = -1.0                   # never equals a real node id or candidate


def _build_program(u_pad, n_dve, cand):
    """Build the SPMD Bass program; candidate ids are baked as immediates."""
    import concourse.bass as bass
    import concourse.mybir as mybir

    AF = mybir.ActivationFunctionType
    ALU = mybir.AluOpType

    # ---- parameter pack layout (one [128, PF] f32 tensor) ----
    C_ONES = 0          # 1.0 in all 128 rows (matmul rhs for partition reduce)
    C_AZB = 1           # -(Lz_top^T bz + Lz_b)   (rows 0:64)
    C_AHB = 2           # Lh_top^T bh + Lh_b      (rows 0:64)
    C_B1P = 3           # (b1 - rmean)*bng + beta (rows 0:64)
    C_B2 = 4            # b2 (row 0)
    C_MULT = 5          # candidate multiplicity weights (rows 0:u_pad)
    C_ROW1 = 6                   # ones_row: 1.0s in row 0, 128 cols
    C_NCB = C_ROW1 + 128         # -cand[j] broadcast down all 128 rows [128, u_pad]
    C_XG = C_NCB + u_pad         # x rows of candidates [u_pad, 64]
    C_AZ = C_XG + 64             # Wz @ Lz_top [64, 64]
    C_AH = C_AZ + 64             # Wh @ Lh_top [64, 64]
    C_W1P = C_AH + 64            # W1 * bng [64, 64]
    C_W2 = C_W1P + 64            # W2 [64, 1]
    PF = C_W2 + 1

    nc = bass.Bass()
    f32 = mybir.dt.float32

    dstv = nc.declare_dram_parameter("dstv", [PART, FREE], f32, isOutput=False)
    pp = nc.declare_dram_parameter("pp", [PART, PF], f32, isOutput=False)
    out = nc.declare_dram_parameter("out", [1, 1], f32, isOutput=True)
    if DEBUG:
        dbg = nc.declare_dram_parameter("dbg", [PART, 16], f32, isOutput=True)

    cc_in = nc.dram_tensor("cc_in", [u_pad, 1], f32)
    cc_out = nc.dram_tensor("cc_out", [u_pad, 1], f32)

    dve_set = list(range(n_dve))
    act_set = list(range(n_dve, u_pad))

    from contextlib import ExitStack

    with ExitStack() as ctx:
        ec = ctx.enter_context
        dst_t = ec(nc.sbuf_tensor("dst_t", [PART, FREE], f32))
        scr = ec(nc.sbuf_tensor("scr", [PART, FREE], f32))
        usq = ec(nc.sbuf_tensor("usq", [PART, FREE], f32))
        ind = ec(nc.sbuf_tensor("ind", [PART, FREE], f32))
        scr2 = ec(nc.sbuf_tensor("scr2", [PART, FREE], f32))
        p_sb = ec(nc.sbuf_tensor("p_sb", [PART, PF], f32))
        cntp = ec(nc.sbuf_tensor("cntp", [PART, u_pad], f32))
        cnt_loc = ec(nc.sbuf_tensor("cnt_loc", [u_pad, 1], f32))
        cnt_tot = ec(nc.sbuf_tensor("cnt_tot", [u_pad, 1], f32))
        s_deg = ec(nc.sbuf_tensor("s_deg", [u_pad, 1], f32))
        dinv = ec(nc.sbuf_tensor("dinv", [u_pad, 1], f32))
        w_col = ec(nc.sbuf_tensor("w_col", [u_pad, 1], f32))
        g_sb = ec(nc.sbuf_tensor("g_sb", [HD, 1], f32))
        zm_sb = ec(nc.sbuf_tensor("zm_sb", [HD, 1], f32))
        ht_sb = ec(nc.sbuf_tensor("ht_sb", [HD, 1], f32))
        htr_sb = ec(nc.sbuf_tensor("htr_sb", [HD, 1], f32))
        y_sb = ec(nc.sbuf_tensor("y_sb", [HD, 1], f32))
        yr_sb = ec(nc.sbuf_tensor("yr_sb", [HD, 1], f32))
        o_sb = ec(nc.sbuf_tensor("o_sb", [1, 1], f32))
        # PSUM (8 banks exist); reuse is serialized by the tok chain
        psB = ec(nc.psum_tensor("psB", [PART, 1], f32))
        psC = ec(nc.psum_tensor("psC", [PART, 1], f32))
        psD = ec(nc.psum_tensor("psD", [PART, 1], f32))
        dsem = ec(nc.semaphore("dsem"))    # input DMAs (16: dst, 32: params)
        csem = ec(nc.semaphore("csem"))    # count loops done (2)
        rsem = ec(nc.semaphore("rsem"))    # partition-reduce matmul done
        lsem = ec(nc.semaphore("lsem"))    # cnt_loc in sbuf
        ccs = ec(nc.semaphore("ccs"))      # collective done
        psem = ec(nc.semaphore("psem"))    # gpsimd DMA (16)
        gsem = ec(nc.semaphore("gsem"))    # cnt_tot in sbuf (16)
        tok = ec(nc.semaphore("tok"))      # epilogue chain
        osem = ec(nc.semaphore("osem"))    # output ready in sbuf
        block = ec(nc.Block())

        ps_cnt = psB[0:u_pad, :]           # [u, 1]  summed partial counts
        ps_d0 = psC[0:u_pad, :]            # [u, 1]  dinv0 broadcast
        ps_g = psB[0:HD, :]                # [64, 1] (ps_cnt consumed by then)
        ps_zp = psC[0:HD, :]               # [64, 1] (ps_d0 consumed by then)
        ps_hp = psD[0:HD, :]               # [64, 1]
        ps_y1 = psB[0:HD, :]               # [64, 1] (ps_g consumed by then)
        ps_o = psD[0:1, :]                 # [1, 1]  (ps_hp consumed by then)

        @block.sync
        def _(sync):
            sync.dma_start(dst_t[:, :], dstv[:, :]).then_inc(dsem, 16)
            sync.dma_start(p_sb[:, :], pp[:, :]).then_inc(dsem, 16)
            sync.wait_ge(osem, 1)
            sync.dma_start(out[:, :], o_sb[:, :]).then_inc(dsem, 16)
            if DEBUG:
                with nc.allow_non_contiguous_dma(reason="debug dumps"):
                    for c, t in enumerate([
                        cnt_loc, cnt_tot, s_deg, dinv, w_col, g_sb, zm_sb,
                        ht_sb, htr_sb, y_sb, yr_sb,
                    ]):
                        sync.dma_start(
                            dbg[0:t.shape[0], c:c + 1], t[:, :]
                        ).then_inc(dsem, 16)

        @block.tensor
        def _(pe):
            pe.wait_ge(dsem, 32)
            pe.wait_ge(csem, 2)
            pe.matmul(ps_cnt, cntp[:, :], p_sb[:, C_ONES:C_ONES + 1]).then_inc(
                rsem, 1
            )
            # epilogue matmuls
            pe.wait_ge(tok, 2)  # dinv ready
            pe.matmul(
                ps_d0, p_sb[0:1, C_ROW1:C_ROW1 + u_pad], dinv[0:1, 0:1]
            ).then_inc(tok, 1)  # -> 3
            pe.wait_ge(tok, 4)
            pe.matmul(ps_g, p_sb[0:u_pad, C_XG:C_XG + HD], w_col[:, :]).then_inc(
                tok, 1
            )  # -> 5
            pe.wait_ge(tok, 6)
            pe.matmul(ps_zp, p_sb[0:HD, C_AZ:C_AZ + HD], g_sb[:, :]).then_inc(
                tok, 1
            )  # -> 7
            pe.matmul(ps_hp, p_sb[0:HD, C_AH:C_AH + HD], g_sb[:, :]).then_inc(
                tok, 1
            )  # -> 8
            pe.wait_ge(tok, 12)
            pe.matmul(ps_y1, p_sb[0:HD, C_W1P:C_W1P + HD], y_sb[:, :]).then_inc(
                tok, 1
            )  # -> 13
            pe.wait_ge(tok, 14)
            pe.matmul(ps_o, p_sb[0:HD, C_W2:C_W2 + 1], yr_sb[:, :]).then_inc(
                tok, 1
            )  # -> 15

        @block.scalar
        def _(act):
            act.wait_ge(dsem, 16)
            for i, j in enumerate(act_set):
                u_t = usq if i % 2 == 0 else ind  # double buffer the |d| tile
                act.activation(
                    u_t[:, :], dst_t[:, :], AF.Abs,
                    bias=p_sb[:, C_NCB + j:C_NCB + j + 1], scale=1.0,
                )
                last = act.activation(
                    scr2[:, :], u_t[:, :], AF.Relu,
                    bias=1.0, scale=-1.0,
                    accum_out=cntp[:, j:j + 1],
                )
            last.then_inc(csem, 1)
            act.wait_ge(rsem, 1)
            act.copy(cnt_loc[:, :], ps_cnt).then_inc(lsem, 1)
            # ---- epilogue ----
            act.wait_ge(gsem, 16)
            act.activation(
                s_deg[:, :], cnt_tot[:, :], AF.Sqrt, bias=1.0, scale=1.0
            ).then_inc(tok, 1)  # -> 1
            act.wait_ge(tok, 5)
            act.copy(g_sb[:, :], ps_g).then_inc(tok, 1)  # -> 6
            act.wait_ge(tok, 8)
            act.activation(
                zm_sb[:, :], ps_zp, AF.Sigmoid,
                bias=p_sb[0:HD, C_AZB:C_AZB + 1], scale=-1.0,
            ).then_inc(tok, 1)  # -> 9   zm = 1 - sigmoid(zpre)
            act.activation(
                ht_sb[:, :], ps_hp, AF.Tanh,
                bias=p_sb[0:HD, C_AHB:C_AHB + 1], scale=1.0,
            ).then_inc(tok, 1)  # -> 10
            act.activation(
                htr_sb[:, :], ht_sb[:, :], AF.Relu, bias=0.0, scale=1.0
            ).then_inc(tok, 1)  # -> 11  relu(Ht); zm>0 so zm*relu(Ht)=relu(zm*Ht)
            act.wait_ge(tok, 13)
            act.activation(
                yr_sb[:, :], ps_y1, AF.Relu,
                bias=p_sb[0:HD, C_B1P:C_B1P + 1], scale=1.0,
            ).then_inc(tok, 1)  # -> 14
            act.wait_ge(tok, 15)
            act.activation(
                o_sb[:, :], ps_o, AF.Identity,
                bias=p_sb[0:1, C_B2:C_B2 + 1], scale=1.0,
            ).then_inc(osem, 1)

        @block.vector
        def _(dve):
            dve.wait_ge(dsem, 16)
            for j in dve_set:
                last = dve.tensor_scalar(
                    scr[:, :],
                    dst_t[:, :],
                    float(cand[j]),
                    None,
                    ALU.is_equal,
                    ALU.add,
                    accum_out=cntp[:, j:j + 1],
                )
            last.then_inc(csem, 1)
            # ---- epilogue ----
            dve.wait_ge(tok, 1)
            dve.reciprocal(dinv[:, :], s_deg[:, :]).then_inc(tok, 1)  # -> 2
            dve.wait_ge(tok, 3)  # ps_d0 ready AND own recip retired
            dve.scalar_tensor_tensor(
                w_col[:, :], dinv[:, :], p_sb[0:u_pad, C_MULT:C_MULT + 1],
                ps_d0, ALU.mult, ALU.mult,
            ).then_inc(tok, 1)  # -> 4   w = dinv*mult*dinv0
            dve.wait_ge(tok, 11)
            dve.tensor_tensor(
                y_sb[:, :], zm_sb[:, :], htr_sb[:, :], ALU.mult
            ).then_inc(tok, 1)  # -> 12

        @block.gpsimd
        def _(gp):
            gp.wait_ge(lsem, 1)
            gp.dma_start(cc_in[:, :], cnt_loc[:, :]).then_inc(psem, 16)
            gp.wait_ge(psem, 16)
            gp.collective_compute(
                "AllReduce",
                mybir.AluOpType.add,
                replica_groups=[list(range(NCORES))],
                ins=[cc_in[:, :].opt()],
                outs=[cc_out[:, :].opt()],
            ).then_inc(ccs, 1)
            gp.wait_ge(ccs, 1)
            gp.dma_start(cnt_tot[:, :], cc_out[:, :]).then_inc(gsem, 16)

    layout = dict(
        C_ONES=C_ONES, C_AZB=C_AZB, C_AHB=C_AHB, C_B1P=C_B1P, C_B2=C_B2,
        C_MULT=C_MULT, C_ROW1=C_ROW1, C_NCB=C_NCB, C_XG=C_XG, C_AZ=C_AZ, C_AH=C_AH,
        C_W1P=C_W1P, C_W2=C_W2, PF=PF,
    )
    return nc, layout


def _prepare(inputs):
    """Host-side preprocessing: find node 2's in-edges, pack params, shard dst."""
    x = np.asarray(inputs["x"], np.float32)
    src = np.asarray(inputs["src"])
    dst = np.asarray(inputs["dst"])

    pos = np.flatnonzero(dst == 2)
    srcs = src[pos]
    uniq, mult = np.unique(srcs, return_counts=True)
    # slot 0 = node 2 itself (for deg2 / the self loop term); then unique sources
    n_slots = 1 + len(uniq)
    u_pad = max(8, -(-(n_slots + 1) // 4) * 4)
    assert n_slots <= 120, f"unexpectedly many in-edges at node 2: {n_slots}"

    cand = np.full(u_pad, -5.0, np.float32)
    multv = np.zeros(u_pad, np.float32)
    cand[0] = 2.0
    multv[0] = 1.0
    cand[1:n_slots] = uniq.astype(np.float32)
    multv[1:n_slots] = mult.astype(np.float32)

    xg = np.zeros((u_pad, HD), np.float32)
    xg[0] = x[2]
    if len(uniq):
        xg[1:n_slots] = x[uniq]

    n_dve = u_pad // 2  # DVE slot (1 op) ~ ACT slot (2 ops) in cost

    nc, L = _build_program(u_pad, n_dve, cand)

    f32 = np.float32
    Wz = np.asarray(inputs["Wz"], f32)
    Wh = np.asarray(inputs["Wh"], f32)
    bz = np.asarray(inputs["bz"], f32)
    bh = np.asarray(inputs["bh"], f32)
    Lz = np.asarray(inputs["Lz_W"], f32)[:HD]
    Lh = np.asarray(inputs["Lh_W"], f32)[:HD]
    Lzb = np.asarray(inputs["Lz_b"], f32)
    Lhb = np.asarray(inputs["Lh_b"], f32)
    W1 = np.asarray(inputs["W1"], f32)
    b1 = np.asarray(inputs["b1"], f32)
    rmean = np.asarray(inputs["rmean"], f32)
    rvar = np.asarray(inputs["rvar"], np.float64)
    gamma = np.asarray(inputs["gamma"], np.float64)
    beta = np.asarray(inputs["beta"], f32)
    bng = (gamma / np.sqrt(rvar + BN_EPS)).astype(f32)

    Az = (Wz @ Lz).astype(f32)
    Ah = (Wh @ Lh).astype(f32)
    azb_neg = -(Lz.T @ bz + Lzb).astype(f32)
    ahb = (Lh.T @ bh + Lhb).astype(f32)
    W1p = (W1 * bng[None, :]).astype(f32)
    b1p = ((b1 - rmean) * bng + beta).astype(f32)

    PF = L["PF"]
    P = np.zeros((PART, PF), f32)
    P[:, L["C_ONES"]] = 1.0
    P[0:HD, L["C_AZB"]] = azb_neg
    P[0:HD, L["C_AHB"]] = ahb
    P[0:HD, L["C_B1P"]] = b1p
    P[0, L["C_B2"]] = np.asarray(inputs["b2"], f32)[0]
    P[0:u_pad, L["C_MULT"]] = multv
    P[0, L["C_ROW1"]:L["C_ROW1"] + 128] = 1.0
    P[:, L["C_NCB"]:L["C_NCB"] + u_pad] = -cand[None, :]
    P[0:u_pad, L["C_XG"]:L["C_XG"] + HD] = xg
    P[0:HD, L["C_AZ"]:L["C_AZ"] + HD] = Az
    P[0:HD, L["C_AH"]:L["C_AH"] + HD] = Ah
    P[0:HD, L["C_W1P"]:L["C_W1P"] + HD] = W1p
    P[0:HD, L["C_W2"]] = np.asarray(inputs["W2"], f32)[:, 0]

    dstp = np.full(NCORES * SHARD, PAD_DST, f32)
    dstp[:E] = dst.astype(f32)
    shards = dstp.reshape(NCORES, PART, FREE)

    in_maps = [{"dstv": shards[i], "pp": P} for i in range(NCORES)]
    return nc, in_maps


def _run(inputs, trace=False):
    from concourse.bass_utils import run_bass_kernel_spmd

    nc, in_maps = _prepare(inputs)
    res = run_bass_kernel_spmd(
        nc, in_maps, core_ids=list(range(NCORES)), trace=trace
    )
    out = np.asarray(res.results[0]["out"], np.float32).reshape(1)
    return out, res


def kernel(**inputs):
    out, _ = _run(inputs, trace=False)
    return out
